# revision 112
# baseline (speedup 1.0000x reference)
import sys
sys.path.insert(0, '/opt/trn_rl_repo')
import numpy as np
import ml_dtypes
import concourse.bacc as bacc
import concourse.mybir as mybir
import concourse.tile as tile
from concourse.bass_utils import run_bass_kernel_spmd

F32 = mybir.dt.float32
BF16 = mybir.dt.bfloat16
ALU = mybir.AluOpType
ACTF = mybir.ActivationFunctionType

B, T, H, O = 16, 2048, 512, 512
NB = 2            # batch rows per core
NCORES = 8
TH = 1024         # scan half length
NH = T // TH      # halves per row
LN_EPS = 1e-6

_CACHE = {}


def _build():
    nc = bacc.Bacc(None, target_bir_lowering=False)
    xin = nc.declare_dram_parameter("x_t", [NB, H, T], BF16, False)
    Brg = nc.declare_dram_parameter("Brg", [H, H], BF16, False)
    Big = nc.declare_dram_parameter("Big", [H, H], BF16, False)
    Crt = nc.declare_dram_parameter("Crt", [H, H], BF16, False)
    Cin = nc.declare_dram_parameter("Cin", [H, H], BF16, False)
    Mw = nc.declare_dram_parameter("Mw", [H, O], BF16, False)
    Rws = nc.declare_dram_parameter("rows", [2, O], BF16, False)
    Ctp = nc.declare_dram_parameter("Ctab", [H, T], BF16, False)
    Stp = nc.declare_dram_parameter("Stab", [H, T], BF16, False)
    CST = nc.declare_dram_parameter("cst", [128, 4 * 6], F32, False)
    out = nc.declare_dram_parameter("out_t", [NB, O, T], F32, True)

    with tile.TileContext(nc) as tc:
        with tc.tile_pool(name="wpool", bufs=1) as wp, \
             tc.tile_pool(name="upool", bufs=1) as up, \
             tc.tile_pool(name="xp", bufs=2) as xp, \
             tc.tile_pool(name="scr", bufs=1) as scr, \
             tc.tile_pool(name="yp", bufs=4) as yp, \
             tc.tile_pool(name="stp", bufs=1) as stp, \
             tc.tile_pool(name="abp", bufs=2) as abp, \
             tc.tile_pool(name="fp", bufs=2) as fp, \
             tc.tile_pool(name="ps_mm1", bufs=2, space="PSUM") as ps1, \
             tc.tile_pool(name="ps_c", bufs=2, space="PSUM") as ps2, \
             tc.tile_pool(name="ps_st", bufs=2, space="PSUM") as pst, \
             tc.tile_pool(name="ps_mlp", bufs=2, space="PSUM") as ps4:

            # ---- resident weights / constants ----
            brg_t = wp.tile([128, 4 * 512], BF16, tag="brg")
            big_t = wp.tile([128, 4 * 512], BF16, tag="big")
            cr_t = wp.tile([128, 4 * 512], BF16, tag="cr")
            ci_t = wp.tile([128, 4 * 512], BF16, tag="ci")
            m_t = wp.tile([128, 4 * 512], BF16, tag="m")
            c_t = wp.tile([128, 4 * T], BF16, tag="ctab")
            s_t = wp.tile([128, 4 * T], BF16, tag="stab")
            cst_t = wp.tile([128, 4 * 6], F32, tag="cst")
            rows_t = wp.tile([1, 2 * O], BF16, tag="rows")
            ones_t = wp.tile([128, 128], BF16, tag="ones")

            eps_t = wp.tile([128, 1], F32, tag="eps")
            def load_bb_half(half):
                # issue via the Act DGE queue so these overlap the SP-queue
                # x loads on the physical DMA engines
                for (dst, src) in ((brg_t, Brg), (big_t, Big)):
                    nc.scalar.dma_start(
                        out=dst[:, half * 1024:(half + 1) * 1024].rearrange(
                            "p (k n) -> p k n", k=2),
                        in_=src[half * 256:(half + 1) * 256, :].rearrange(
                            "(k p) n -> p k n", p=128))

            load_bb_half(0)
            nc.vector.memset(ones_t[:], 1.0)

            nc.vector.memset(eps_t[:], LN_EPS)

            def load_misc():
                nc.scalar.dma_start(out=cst_t[:], in_=CST[:])

            def load_tables(blk, th):
                # (blk, t-half) pieces on the Pool DGE queue: not gated on the
                # full transfers nor on the SP/Act queue backlogs
                for (dst, src) in ((c_t, Ctp), (s_t, Stp)):
                    nc.gpsimd.dma_start(
                        out=dst[:, blk * T + th * TH:blk * T + (th + 1) * TH],
                        in_=src[blk * 128:(blk + 1) * 128, th * TH:(th + 1) * TH])

            def load_bulk_weights():
                nc.sync.dma_start(
                    out=rows_t[:].rearrange("p (a n) -> p a n", a=2),
                    in_=Rws[:].rearrange("(p a) n -> p a n", p=1))
                for (dst, src) in ((cr_t, Crt), (ci_t, Cin), (m_t, Mw)):
                    nc.sync.dma_start(
                        out=dst[:].rearrange("p (k n) -> p k n", k=4),
                        in_=src[:].rearrange("(k p) n -> p k n", p=128))

            def col(c, blk):
                return cst_t[:, c * 4 + blk:c * 4 + blk + 1]

            # A: ping-pong U/V buffers (half-row granularity, in-place derot)
            Ar = up.tile([128, 2 * 4 * TH], BF16, tag="ar")
            Ai = up.tile([128, 2 * 4 * TH], BF16, tag="ai")
            avr = Ar[:].rearrange("p (q k t) -> p q k t", q=2, k=4)
            avi = Ai[:].rearrange("p (q k t) -> p q k t", q=2, k=4)
            # B: G -> H (in-place rerot), full rows
            Gr = up.tile([128, 4 * NB * T], BF16, tag="gr")
            Gi = up.tile([128, 4 * NB * T], BF16, tag="gi")
            b4r = Gr[:].rearrange("p (k b t) -> p k b t", k=4, b=NB)
            b4i = Gi[:].rearrange("p (k b t) -> p k b t", k=4, b=NB)
            c4 = c_t[:].rearrange("p (k t) -> p k t", k=4)
            s4 = s_t[:].rearrange("p (k t) -> p k t", k=4)

            def asl(av, pp, blk, c0, c1):
                return av[:, pp:pp + 1, blk:blk + 1, c0:c1].squeeze()

            def bsl(b4, blk, b, c0, c1):
                return b4[:, blk:blk + 1, b:b + 1, c0:c1].squeeze()

            def csl(c4v, blk, c0, c1):
                return c4v[:, blk:blk + 1, c0:c1].squeeze()

            def mm1(b, h):
                # ob-major loop order: each h-block's U completes early so the
                # derot for blk 0 starts after ~1/4 of mm1, not all of it
                pp = (2 * b + h) % 2
                xts = []
                for m in range(TH // 512):
                    t0 = h * TH + m * 512
                    xt = xp.tile([128, 4 * 512], BF16, tag="xt")
                    for half in range(2):
                        nc.sync.dma_start(
                            out=xt[:, half * 1024:(half + 1) * 1024].rearrange(
                                "p (k t) -> p k t", k=2),
                            in_=xin[b, half * 256:(half + 1) * 256, t0:t0 + 512].rearrange(
                                "(k p) t -> p k t", p=128))
                    xts.append(xt)
                for ob in range(4):
                    for m in range(TH // 512):
                        for (wt, dst, bcol) in ((brg_t, avr, 1), (big_t, avi, 2)):
                            pm = ps1.tile([128, 512], F32, tag="pm1")
                            for kt in range(4):
                                nc.tensor.matmul(
                                    pm[:], wt[:, kt * 512 + ob * 128:kt * 512 + ob * 128 + 128],
                                    xts[m][:, kt * 512:(kt + 1) * 512],
                                    start=(kt == 0), stop=(kt == 3))
                            slab = asl(dst, pp, ob, m * 512, (m + 1) * 512)
                            nc.scalar.activation(
                                slab, pm[:], ACTF.Identity, bias=col(bcol, ob), scale=1.0)

            def derot(b, h):
                # in-place: U -> V = e^{-i theta t} U; blk-pair ops (3-D APs)
                pp = (2 * b + h) % 2
                g0 = h * TH
                for pr in range(2):
                    k0, k1 = 2 * pr, 2 * pr + 2
                    ur = avr[:, pp:pp + 1, k0:k1, :].squeeze()
                    ui = avi[:, pp:pp + 1, k0:k1, :].squeeze()
                    cc = c4[:, k0:k1, g0:g0 + TH]
                    ss = s4[:, k0:k1, g0:g0 + TH]
                    t1 = scr.tile([128, 2 * TH], BF16, tag="sc1")
                    t2 = scr.tile([128, 2 * TH], BF16, tag="sc2")
                    t1v = t1[:].rearrange("p (a t) -> p a t", a=2)
                    t2v = t2[:].rearrange("p (a t) -> p a t", a=2)
                    nc.vector.tensor_tensor(t1v, ur, ss, ALU.mult)
                    nc.vector.tensor_tensor(t2v, ui, ss, ALU.mult)
                    nc.vector.tensor_tensor(ur, ur, cc, ALU.mult)
                    nc.vector.tensor_tensor(ui, ui, cc, ALU.mult)
                    nc.vector.tensor_tensor(ur, ur, t2v, ALU.add)
                    nc.vector.tensor_tensor(ui, ui, t1v, ALU.subtract)

            def scan(b, h, carries):
                pp = (2 * b + h) % 2
                g0 = h * TH
                for blk in range(4):
                    vr = asl(avr, pp, blk, 0, TH)
                    vi = asl(avi, pp, blk, 0, TH)
                    gr = bsl(b4r, blk, b, g0, g0 + TH)
                    gi = bsl(b4i, blk, b, g0, g0 + TH)
                    rho = col(0, blk).broadcast_to([128, TH])
                    if h == 0:
                        init_r, init_i = 0.0, 0.0
                    else:
                        car_r, car_i = carries
                        init_r = car_r[:, blk:blk + 1]
                        init_i = car_i[:, blk:blk + 1]
                    nc.vector.tensor_tensor_scan(gr, rho, vr, init_r, ALU.mult, ALU.add)
                    nc.vector.tensor_tensor_scan(gi, rho, vi, init_i, ALU.mult, ALU.add)

            def carry_save(b):
                # snapshot G[:, TH-1] per blk before rerot(b,0) clobbers it
                car_r = stp.tile([128, 4], BF16, tag="carR", bufs=2, name=f"carR_{b}")
                car_i = stp.tile([128, 4], BF16, tag="carI", bufs=2, name=f"carI_{b}")
                src_r = b4r[:, :, b:b + 1, TH - 1:TH].squeeze()
                src_i = b4i[:, :, b:b + 1, TH - 1:TH].squeeze()
                nc.vector.tensor_copy(car_r[:], src_r)
                nc.vector.tensor_copy(car_i[:], src_i)
                return car_r, car_i

            def rerot(b, h):
                # in-place: G -> H = e^{+i theta t} G; blk-pair ops (3-D APs)
                g0 = h * TH
                for pr in range(2):
                    k0, k1 = 2 * pr, 2 * pr + 2
                    gr = b4r[:, k0:k1, b:b + 1, g0:g0 + TH].squeeze()
                    gi = b4i[:, k0:k1, b:b + 1, g0:g0 + TH].squeeze()
                    cc = c4[:, k0:k1, g0:g0 + TH]
                    ss = s4[:, k0:k1, g0:g0 + TH]
                    t1 = scr.tile([128, 2 * TH], BF16, tag="sc1")
                    t2 = scr.tile([128, 2 * TH], BF16, tag="sc2")
                    t1v = t1[:].rearrange("p (a t) -> p a t", a=2)
                    t2v = t2[:].rearrange("p (a t) -> p a t", a=2)
                    nc.vector.tensor_tensor(t1v, gr, ss, ALU.mult)
                    nc.vector.tensor_tensor(t2v, gi, ss, ALU.mult)
                    nc.vector.tensor_tensor(gr, gr, cc, ALU.mult)
                    nc.vector.tensor_tensor(gi, gi, cc, ALU.mult)
                    nc.vector.tensor_tensor(gr, gr, t2v, ALU.subtract)
                    nc.vector.tensor_tensor(gi, gi, t1v, ALU.add)

            def passA(b, h, m, w=512, toff=0):
                # C readout + LN stats chain; returns tiles passB needs
                t0 = h * TH + m * 512 + toff
                ys = []
                for ob in range(4):
                    p2 = ps2.tile([128, 512], F32, tag="p2")
                    for kt in range(4):
                        nc.tensor.matmul(
                            p2[:, 0:w], cr_t[:, kt * 512 + ob * 128:kt * 512 + ob * 128 + 128],
                            bsl(b4r, kt, b, t0, t0 + w), start=(kt == 0), stop=False)
                    for kt in range(4):
                        nc.tensor.matmul(
                            p2[:, 0:w], ci_t[:, kt * 512 + ob * 128:kt * 512 + ob * 128 + 128],
                            bsl(b4i, kt, b, t0, t0 + w), start=False, stop=(kt == 3))
                    y = yp.tile([128, 512], BF16, tag="y", bufs=16)
                    nc.scalar.activation(y[:, 0:w], p2[:, 0:w], ACTF.Identity, bias=col(3, ob), scale=1.0)
                    ys.append(y)
                y2s = []
                for ob in range(4):
                    y2 = yp.tile([128, 512], BF16, tag="y2", bufs=2)
                    nc.scalar.activation(y2[:, 0:w], ys[ob][:, 0:w], ACTF.Square)
                    y2s.append(y2)
                s1 = pst.tile([1, 512], F32, tag="pstat", name="s1")
                s2 = pst.tile([1, 512], F32, tag="pstat", name="s2")
                for ob in range(4):
                    nc.tensor.matmul(s1[:, 0:w], ones_t[:, 0:1], ys[ob][:, 0:w],
                                     start=(ob == 0), stop=(ob == 3))
                for ob in range(4):
                    nc.tensor.matmul(s2[:, 0:w], ones_t[:, 0:1], y2s[ob][:, 0:w],
                                     start=(ob == 0), stop=(ob == 3))
                mean = stp.tile([1, 512], F32, tag="mean")
                ms = stp.tile([1, 512], F32, tag="ms")
                var = stp.tile([1, 512], F32, tag="var")
                nc.scalar.activation(mean[:, 0:w], s1[:, 0:w], ACTF.Copy, scale=1.0 / H)
                nc.scalar.activation(ms[:, 0:w], s1[:, 0:w], ACTF.Square, scale=1.0 / H)
                nc.vector.scalar_tensor_tensor(var[:, 0:w], s2[:, 0:w], 1.0 / H, ms[:, 0:w], ALU.mult, ALU.subtract)
                # in-place: var -> sd
                nc.scalar.activation(var[:, 0:w], var[:, 0:w], ACTF.Sqrt, bias=eps_t[0:1, :])
                # reciprocal emits bf16 directly: one fewer chain hop
                Ab_h = stp.tile([1, 512], BF16, tag="Abh", name=f"Abh_{b}_{h}_{m}_{toff}", bufs=3)
                Bb_h = stp.tile([1, 512], BF16, tag="Bbh", name=f"Bbh_{b}_{h}_{m}_{toff}", bufs=3)
                with nc.allow_low_precision(reason="LN scale in bf16; 0.4% on 2e-2 budget"):
                    nc.vector.reciprocal(Ab_h[:, 0:w], var[:, 0:w])
                    nc.vector.scalar_tensor_tensor(Bb_h[:, 0:w], mean[:, 0:w], -1.0, Ab_h[:, 0:w], ALU.mult, ALU.mult)
                return ys, Ab_h, Bb_h

            def passB(b, h, m, st, w=512, toff=0):
                t0 = h * TH + m * 512 + toff
                ys, Ab_h, Bb_h = st
                Ab = abp.tile([128, 512], BF16, tag="Ab")
                nc.gpsimd.partition_broadcast(Ab[:, 0:w], Ab_h[:, 0:w])
                yss = []
                for kt in range(4):
                    ysc = yp.tile([128, 512], BF16, tag="ysc", bufs=4)
                    nc.vector.tensor_tensor(ysc[:, 0:w], ys[kt][:, 0:w], Ab[:, 0:w], ALU.mult)
                    yss.append(ysc)
                for ob in range(4):
                    p4 = ps4.tile([128, 512], F32, tag="p4")
                    for kt in range(4):
                        nc.tensor.matmul(
                            p4[:, 0:w], m_t[:, kt * 512 + ob * 128:kt * 512 + ob * 128 + 128],
                            yss[kt][:, 0:w], start=(kt == 0), stop=False)
                    nc.tensor.matmul(p4[:, 0:w], rows_t[0:1, ob * 128:ob * 128 + 128],
                                     Bb_h[:, 0:w], start=False, stop=True)
                    of = fp.tile([128, 512], F32, tag="of")
                    # c7 (constant per channel) rides the evac bias port
                    nc.scalar.activation(of[:, 0:w], p4[:, 0:w], ACTF.Identity,
                                         bias=col(5, ob), scale=1.0)
                    nc.gpsimd.dma_start(out=out[b, ob * 128:(ob + 1) * 128, t0:t0 + w], in_=of[:, 0:w])

            def dsr(b, h, carries=None):
                derot(b, h)
                scan(b, h, carries)

            def phase2_row(b, look=1):
                # interleave passA/passB with `look`-unit lookahead so PE
                # stays fed during each unit's LN scalar chain
                units = [(h, m) for h in range(NH) for m in range(TH // 512)]
                st = {}
                for i, (h, m) in enumerate(units):
                    st[i] = passA(b, h, m)
                    if i >= look:
                        ph, pm_ = units[i - look]
                        passB(b, ph, pm_, st.pop(i - look))
                for i in range(len(units) - look, len(units)):
                    passB(b, *units[i], st.pop(i))

            # Emission = per-engine execution order (in-order queues).  Each
            # half's rerot runs right after its scan (carry snapshot unlocks
            # the in-place overwrite), and phase2 units zip between row-1
            # scan/rerot stages so the DVE stream never drains.
            U = [(h, m) for h in range(NH) for m in range(TH // 512)]
            load_bb_half(1)
            load_misc()
            # PE pstate warm-up: dummy matmuls during DMA lead-ins / rerot
            # waits keep the tensor engine clock ramped. wide=True spans long
            # waits with 512-cycle dummies (rhs contents are irrelevant).
            def pe_warm(n, wide=False):
                if wide:
                    warm = pst.tile([1, 512], F32, tag="pstat", name="warmw")
                    for _ in range(n):
                        nc.tensor.matmul(warm[:], ones_t[:, 0:1], brg_t[:, 0:512],
                                         start=True, stop=True)
                else:
                    warm = pst.tile([1, 128], F32, tag="pstat", name="warm")
                    for _ in range(n):
                        nc.tensor.matmul(warm[:], ones_t[:, 0:1], ones_t[:, 0:128],
                                         start=True, stop=True)

            pe_warm(55)
            mm1(0, 0)
            for blk in range(4):
                load_tables(blk, 0)
            dsr(0, 0)
            car0 = carry_save(0)
            rerot(0, 0)
            mm1(0, 1)
            for blk in range(4):
                load_tables(blk, 1)
            load_bulk_weights()
            dsr(0, 1, car0)
            rerot(0, 1)
            mm1(1, 0)
            mm1(1, 1)
            st0 = {}
            st1 = {}
            st0[0] = passA(0, *U[0])
            st0[1] = passA(0, *U[1])
            passB(0, *U[0], st0.pop(0))
            dsr(1, 0)
            car1 = carry_save(1)
            rerot(1, 0)
            pe_warm(8, wide=True)
            st0[2] = passA(0, *U[2])
            pe_warm(35, wide=True)
            passB(0, *U[1], st0.pop(1))
            dsr(1, 1, car1)
            pe_warm(30, wide=True)
            st1[0] = passA(1, *U[0])
            st1[1] = passA(1, *U[1])
            rerot(1, 1)
            st0[3] = passA(0, *U[3])
            passB(0, *U[2], st0.pop(2))
            passB(0, *U[3], st0.pop(3))
            passB(1, *U[0], st1.pop(0))
            pe_warm(15, wide=True)
            st1[2] = passA(1, *U[2])
            passB(1, *U[1], st1.pop(1))
            st1[3] = passA(1, *U[3])
            passB(1, *U[2], st1.pop(2))
            passB(1, *U[3], st1.pop(3))

    nc.compile()
    return nc


def _consts(inputs):
    f64 = np.float64
    theta = np.exp(inputs["theta_log"].astype(np.float32)).astype(f64)
    rho = np.exp(-np.exp(inputs["nu_log"].astype(f64)))
    gamma = np.exp(inputs["gamma_log"].astype(f64))
    tt = np.arange(T, dtype=f64)
    ang = theta[:, None] * tt[None, :]
    Ctab = np.cos(ang)
    Stab = np.sin(ang)
    W1 = inputs["W1"].astype(f64)
    W2 = inputs["W2"].astype(f64)
    W1s = W1 * inputs["ln_scale"].astype(f64)[:, None]
    M = W1s @ W2
    c6 = M.sum(0)
    c7 = (inputs["ln_bias"].astype(f64) @ W1 + inputs["b1"].astype(f64)) @ W2 \
        + inputs["b2"].astype(f64)
    cols = {
        0: rho,
        1: inputs["br"].astype(f64) * gamma,
        2: inputs["bi"].astype(f64) * gamma,
        3: (inputs["cr"] - inputs["ci"]).astype(f64),
        4: c6,
        5: c7,
    }
    cst = np.zeros((128, 4 * 6), np.float32)
    for c, v in cols.items():
        for blk in range(4):
            cst[:, c * 4 + blk] = v[blk * 128:(blk + 1) * 128].astype(np.float32)
    rows = np.stack([c6, c7]).astype(np.float32)  # [2, O]
    return cst, gamma, M, Ctab, Stab, rows


def _shared_inputs(inputs):
    cst, gamma, M, Ctab, Stab, rows = _consts(inputs)
    bf = ml_dtypes.bfloat16
    g32 = gamma.astype(np.float32)
    return dict(
        Brg=(inputs["Br"] * g32[None, :]).astype(bf),
        Big=(inputs["Bi"] * g32[None, :]).astype(bf),
        Crt=inputs["Cr"].astype(bf),
        Cin=(-inputs["Ci"]).astype(bf),
        Mw=M.astype(np.float32).astype(bf),
        rows=rows.astype(bf),
        Ctab=Ctab.astype(np.float32).astype(bf),
        Stab=Stab.astype(np.float32).astype(bf),
        cst=cst)


def _core_inputs(inputs, i):
    bf = ml_dtypes.bfloat16
    sh = _shared_inputs(inputs)
    xt = np.ascontiguousarray(
        inputs["x"][NB * i:NB * (i + 1)].transpose(0, 2, 1)).astype(bf)
    return dict(x_t=xt, **sh)


def kernel(x, nu_log, theta_log, gamma_log, Br, br, Bi, bi,
           Cr, cr, Ci, ci, ln_scale, ln_bias, W1, b1, W2, b2):
    if "nc" not in _CACHE:
        _CACHE["nc"] = _build()
    nc = _CACHE["nc"]
    inputs = dict(x=x, nu_log=nu_log, theta_log=theta_log, gamma_log=gamma_log,
                  Br=Br, br=br, Bi=Bi, bi=bi, Cr=Cr, cr=cr, Ci=Ci, ci=ci,
                  ln_scale=ln_scale, ln_bias=ln_bias, W1=W1, b1=b1, W2=W2, b2=b2)
    sh = _shared_inputs(inputs)
    bf = ml_dtypes.bfloat16
    xt = np.ascontiguousarray(x.transpose(0, 2, 1)).astype(bf)  # [B, H, T]
    in_maps = []
    for i in range(NCORES):
        in_maps.append(dict(x_t=xt[NB * i:NB * (i + 1)], **sh))
    res = run_bass_kernel_spmd(nc, in_maps, core_ids=list(range(NCORES)))
    out = np.empty((B, T, O), np.float32)
    for i in range(NCORES):
        o = res.results[i]["out_t"]  # [NB, O, T]
        out[NB * i:NB * (i + 1)] = o.transpose(0, 2, 1)
    return out


# revision 115
# speedup vs baseline: 1.0126x; 1.0126x over previous
import sys
sys.path.insert(0, '/opt/trn_rl_repo')
import numpy as np
import ml_dtypes
import concourse.bacc as bacc
import concourse.mybir as mybir
import concourse.tile as tile
from concourse.bass_utils import run_bass_kernel_spmd

F32 = mybir.dt.float32
BF16 = mybir.dt.bfloat16
ALU = mybir.AluOpType
ACTF = mybir.ActivationFunctionType

B, T, H, O = 16, 2048, 512, 512
NB = 2            # batch rows per core
NCORES = 8
TH = 1024         # scan half length
NH = T // TH      # halves per row
LN_EPS = 1e-6

_CACHE = {}


def _build():
    nc = bacc.Bacc(None, target_bir_lowering=False)
    xin = nc.declare_dram_parameter("x_t", [NB, H, T], BF16, False)
    Brg = nc.declare_dram_parameter("Brg", [H, H], BF16, False)
    Big = nc.declare_dram_parameter("Big", [H, H], BF16, False)
    Crt = nc.declare_dram_parameter("Crt", [H, H], BF16, False)
    Cin = nc.declare_dram_parameter("Cin", [H, H], BF16, False)
    Mw = nc.declare_dram_parameter("Mw", [H, O], BF16, False)
    Rws = nc.declare_dram_parameter("rows", [2, O], BF16, False)
    Ctp = nc.declare_dram_parameter("Ctab", [H, T], BF16, False)
    Stp = nc.declare_dram_parameter("Stab", [H, T], BF16, False)
    CST = nc.declare_dram_parameter("cst", [128, 4 * 6], F32, False)
    out = nc.declare_dram_parameter("out_t", [NB, O, T], F32, True)

    with tile.TileContext(nc) as tc:
        with tc.tile_pool(name="wpool", bufs=1) as wp, \
             tc.tile_pool(name="upool", bufs=1) as up, \
             tc.tile_pool(name="xp", bufs=2) as xp, \
             tc.tile_pool(name="scr", bufs=1) as scr, \
             tc.tile_pool(name="yp", bufs=4) as yp, \
             tc.tile_pool(name="stp", bufs=1) as stp, \
             tc.tile_pool(name="abp", bufs=2) as abp, \
             tc.tile_pool(name="fp", bufs=2) as fp, \
             tc.tile_pool(name="ps_mm1", bufs=2, space="PSUM") as ps1, \
             tc.tile_pool(name="ps_c", bufs=2, space="PSUM") as ps2, \
             tc.tile_pool(name="ps_st", bufs=2, space="PSUM") as pst, \
             tc.tile_pool(name="ps_mlp", bufs=2, space="PSUM") as ps4:

            # ---- resident weights / constants ----
            brg_t = wp.tile([128, 4 * 512], BF16, tag="brg")
            big_t = wp.tile([128, 4 * 512], BF16, tag="big")
            cr_t = wp.tile([128, 4 * 512], BF16, tag="cr")
            ci_t = wp.tile([128, 4 * 512], BF16, tag="ci")
            m_t = wp.tile([128, 4 * 512], BF16, tag="m")
            c_t = wp.tile([128, 4 * T], BF16, tag="ctab")
            s_t = wp.tile([128, 4 * T], BF16, tag="stab")
            cst_t = wp.tile([128, 4 * 6], F32, tag="cst")
            rows_t = wp.tile([1, 2 * O], BF16, tag="rows")
            ones_t = wp.tile([128, 128], BF16, tag="ones")

            eps_t = wp.tile([128, 1], F32, tag="eps")
            def load_bb_half(half):
                # issue via the Act DGE queue so these overlap the SP-queue
                # x loads on the physical DMA engines
                for (dst, src) in ((brg_t, Brg), (big_t, Big)):
                    nc.scalar.dma_start(
                        out=dst[:, half * 1024:(half + 1) * 1024].rearrange(
                            "p (k n) -> p k n", k=2),
                        in_=src[half * 256:(half + 1) * 256, :].rearrange(
                            "(k p) n -> p k n", p=128))

            load_bb_half(0)
            nc.vector.memset(ones_t[:], 1.0)

            nc.vector.memset(eps_t[:], LN_EPS)

            def load_misc():
                nc.scalar.dma_start(out=cst_t[:], in_=CST[:])

            def load_tables(blk, th):
                # (blk, t-half) pieces on the Pool DGE queue: not gated on the
                # full transfers nor on the SP/Act queue backlogs
                for (dst, src) in ((c_t, Ctp), (s_t, Stp)):
                    nc.gpsimd.dma_start(
                        out=dst[:, blk * T + th * TH:blk * T + (th + 1) * TH],
                        in_=src[blk * 128:(blk + 1) * 128, th * TH:(th + 1) * TH])

            def load_bulk_weights():
                nc.sync.dma_start(
                    out=rows_t[:].rearrange("p (a n) -> p a n", a=2),
                    in_=Rws[:].rearrange("(p a) n -> p a n", p=1))
                for (dst, src) in ((cr_t, Crt), (ci_t, Cin), (m_t, Mw)):
                    nc.sync.dma_start(
                        out=dst[:].rearrange("p (k n) -> p k n", k=4),
                        in_=src[:].rearrange("(k p) n -> p k n", p=128))

            def col(c, blk):
                return cst_t[:, c * 4 + blk:c * 4 + blk + 1]

            # A: ping-pong U/V buffers (half-row granularity, in-place derot)
            Ar = up.tile([128, 2 * 4 * TH], BF16, tag="ar")
            Ai = up.tile([128, 2 * 4 * TH], BF16, tag="ai")
            avr = Ar[:].rearrange("p (q k t) -> p q k t", q=2, k=4)
            avi = Ai[:].rearrange("p (q k t) -> p q k t", q=2, k=4)
            # B: G -> H (in-place rerot), full rows
            Gr = up.tile([128, 4 * NB * T], BF16, tag="gr")
            Gi = up.tile([128, 4 * NB * T], BF16, tag="gi")
            b4r = Gr[:].rearrange("p (k b t) -> p k b t", k=4, b=NB)
            b4i = Gi[:].rearrange("p (k b t) -> p k b t", k=4, b=NB)
            c4 = c_t[:].rearrange("p (k t) -> p k t", k=4)
            s4 = s_t[:].rearrange("p (k t) -> p k t", k=4)

            def asl(av, pp, blk, c0, c1):
                return av[:, pp:pp + 1, blk:blk + 1, c0:c1].squeeze()

            def bsl(b4, blk, b, c0, c1):
                return b4[:, blk:blk + 1, b:b + 1, c0:c1].squeeze()

            def csl(c4v, blk, c0, c1):
                return c4v[:, blk:blk + 1, c0:c1].squeeze()

            def mm1(b, h):
                # ob-major loop order: each h-block's U completes early so the
                # derot for blk 0 starts after ~1/4 of mm1, not all of it
                pp = (2 * b + h) % 2
                xts = []
                for m in range(TH // 512):
                    t0 = h * TH + m * 512
                    xt = xp.tile([128, 4 * 512], BF16, tag="xt")
                    for half in range(2):
                        nc.sync.dma_start(
                            out=xt[:, half * 1024:(half + 1) * 1024].rearrange(
                                "p (k t) -> p k t", k=2),
                            in_=xin[b, half * 256:(half + 1) * 256, t0:t0 + 512].rearrange(
                                "(k p) t -> p k t", p=128))
                    xts.append(xt)
                for ob in range(4):
                    for m in range(TH // 512):
                        for (wt, dst, bcol) in ((brg_t, avr, 1), (big_t, avi, 2)):
                            pm = ps1.tile([128, 512], F32, tag="pm1")
                            for kt in range(4):
                                nc.tensor.matmul(
                                    pm[:], wt[:, kt * 512 + ob * 128:kt * 512 + ob * 128 + 128],
                                    xts[m][:, kt * 512:(kt + 1) * 512],
                                    start=(kt == 0), stop=(kt == 3))
                            slab = asl(dst, pp, ob, m * 512, (m + 1) * 512)
                            nc.scalar.activation(
                                slab, pm[:], ACTF.Identity, bias=col(bcol, ob), scale=1.0)

            def derot_fine(b, h):
                # per-blk variant: blk k only needs ob-k's mm1 evacs, so the
                # first derot of the kernel starts ~3us earlier
                pp = (2 * b + h) % 2
                g0 = h * TH
                for blk in range(4):
                    ur = asl(avr, pp, blk, 0, TH)
                    ui = asl(avi, pp, blk, 0, TH)
                    cc = csl(c4, blk, g0, g0 + TH)
                    ss = csl(s4, blk, g0, g0 + TH)
                    t1 = scr.tile([128, 2 * TH], BF16, tag="sc1")
                    t2 = scr.tile([128, 2 * TH], BF16, tag="sc2")
                    nc.vector.tensor_tensor(t1[:, 0:TH], ur, ss, ALU.mult)
                    nc.vector.tensor_tensor(t2[:, 0:TH], ui, ss, ALU.mult)
                    nc.vector.tensor_tensor(ur, ur, cc, ALU.mult)
                    nc.vector.tensor_tensor(ui, ui, cc, ALU.mult)
                    nc.vector.tensor_tensor(ur, ur, t2[:, 0:TH], ALU.add)
                    nc.vector.tensor_tensor(ui, ui, t1[:, 0:TH], ALU.subtract)

            def derot(b, h):
                # in-place: U -> V = e^{-i theta t} U; blk-pair ops (3-D APs)
                pp = (2 * b + h) % 2
                g0 = h * TH
                for pr in range(2):
                    k0, k1 = 2 * pr, 2 * pr + 2
                    ur = avr[:, pp:pp + 1, k0:k1, :].squeeze()
                    ui = avi[:, pp:pp + 1, k0:k1, :].squeeze()
                    cc = c4[:, k0:k1, g0:g0 + TH]
                    ss = s4[:, k0:k1, g0:g0 + TH]
                    t1 = scr.tile([128, 2 * TH], BF16, tag="sc1")
                    t2 = scr.tile([128, 2 * TH], BF16, tag="sc2")
                    t1v = t1[:].rearrange("p (a t) -> p a t", a=2)
                    t2v = t2[:].rearrange("p (a t) -> p a t", a=2)
                    nc.vector.tensor_tensor(t1v, ur, ss, ALU.mult)
                    nc.vector.tensor_tensor(t2v, ui, ss, ALU.mult)
                    nc.vector.tensor_tensor(ur, ur, cc, ALU.mult)
                    nc.vector.tensor_tensor(ui, ui, cc, ALU.mult)
                    nc.vector.tensor_tensor(ur, ur, t2v, ALU.add)
                    nc.vector.tensor_tensor(ui, ui, t1v, ALU.subtract)

            def scan(b, h, carries):
                pp = (2 * b + h) % 2
                g0 = h * TH
                for blk in range(4):
                    vr = asl(avr, pp, blk, 0, TH)
                    vi = asl(avi, pp, blk, 0, TH)
                    gr = bsl(b4r, blk, b, g0, g0 + TH)
                    gi = bsl(b4i, blk, b, g0, g0 + TH)
                    rho = col(0, blk).broadcast_to([128, TH])
                    if h == 0:
                        init_r, init_i = 0.0, 0.0
                    else:
                        car_r, car_i = carries
                        init_r = car_r[:, blk:blk + 1]
                        init_i = car_i[:, blk:blk + 1]
                    nc.vector.tensor_tensor_scan(gr, rho, vr, init_r, ALU.mult, ALU.add)
                    nc.vector.tensor_tensor_scan(gi, rho, vi, init_i, ALU.mult, ALU.add)

            def carry_save(b):
                # snapshot G[:, TH-1] per blk before rerot(b,0) clobbers it
                car_r = stp.tile([128, 4], BF16, tag="carR", bufs=2, name=f"carR_{b}")
                car_i = stp.tile([128, 4], BF16, tag="carI", bufs=2, name=f"carI_{b}")
                src_r = b4r[:, :, b:b + 1, TH - 1:TH].squeeze()
                src_i = b4i[:, :, b:b + 1, TH - 1:TH].squeeze()
                nc.vector.tensor_copy(car_r[:], src_r)
                nc.vector.tensor_copy(car_i[:], src_i)
                return car_r, car_i

            def rerot(b, h):
                # in-place: G -> H = e^{+i theta t} G; blk-pair ops (3-D APs)
                g0 = h * TH
                for pr in range(2):
                    k0, k1 = 2 * pr, 2 * pr + 2
                    gr = b4r[:, k0:k1, b:b + 1, g0:g0 + TH].squeeze()
                    gi = b4i[:, k0:k1, b:b + 1, g0:g0 + TH].squeeze()
                    cc = c4[:, k0:k1, g0:g0 + TH]
                    ss = s4[:, k0:k1, g0:g0 + TH]
                    t1 = scr.tile([128, 2 * TH], BF16, tag="sc1")
                    t2 = scr.tile([128, 2 * TH], BF16, tag="sc2")
                    t1v = t1[:].rearrange("p (a t) -> p a t", a=2)
                    t2v = t2[:].rearrange("p (a t) -> p a t", a=2)
                    nc.vector.tensor_tensor(t1v, gr, ss, ALU.mult)
                    nc.vector.tensor_tensor(t2v, gi, ss, ALU.mult)
                    nc.vector.tensor_tensor(gr, gr, cc, ALU.mult)
                    nc.vector.tensor_tensor(gi, gi, cc, ALU.mult)
                    nc.vector.tensor_tensor(gr, gr, t2v, ALU.subtract)
                    nc.vector.tensor_tensor(gi, gi, t1v, ALU.add)

            def passA(b, h, m, w=512, toff=0):
                # C readout + LN stats chain; returns tiles passB needs
                t0 = h * TH + m * 512 + toff
                ys = []
                for ob in range(4):
                    p2 = ps2.tile([128, 512], F32, tag="p2")
                    for kt in range(4):
                        nc.tensor.matmul(
                            p2[:, 0:w], cr_t[:, kt * 512 + ob * 128:kt * 512 + ob * 128 + 128],
                            bsl(b4r, kt, b, t0, t0 + w), start=(kt == 0), stop=False)
                    for kt in range(4):
                        nc.tensor.matmul(
                            p2[:, 0:w], ci_t[:, kt * 512 + ob * 128:kt * 512 + ob * 128 + 128],
                            bsl(b4i, kt, b, t0, t0 + w), start=False, stop=(kt == 3))
                    y = yp.tile([128, 512], BF16, tag="y", bufs=16)
                    nc.scalar.activation(y[:, 0:w], p2[:, 0:w], ACTF.Identity, bias=col(3, ob), scale=1.0)
                    ys.append(y)
                y2s = []
                for ob in range(4):
                    y2 = yp.tile([128, 512], BF16, tag="y2", bufs=2)
                    nc.scalar.activation(y2[:, 0:w], ys[ob][:, 0:w], ACTF.Square)
                    y2s.append(y2)
                s1 = pst.tile([1, 512], F32, tag="pstat", name="s1")
                s2 = pst.tile([1, 512], F32, tag="pstat", name="s2")
                for ob in range(4):
                    nc.tensor.matmul(s1[:, 0:w], ones_t[:, 0:1], ys[ob][:, 0:w],
                                     start=(ob == 0), stop=(ob == 3))
                for ob in range(4):
                    nc.tensor.matmul(s2[:, 0:w], ones_t[:, 0:1], y2s[ob][:, 0:w],
                                     start=(ob == 0), stop=(ob == 3))
                mean = stp.tile([1, 512], F32, tag="mean")
                ms = stp.tile([1, 512], F32, tag="ms")
                var = stp.tile([1, 512], F32, tag="var")
                nc.scalar.activation(mean[:, 0:w], s1[:, 0:w], ACTF.Copy, scale=1.0 / H)
                nc.scalar.activation(ms[:, 0:w], s1[:, 0:w], ACTF.Square, scale=1.0 / H)
                nc.vector.scalar_tensor_tensor(var[:, 0:w], s2[:, 0:w], 1.0 / H, ms[:, 0:w], ALU.mult, ALU.subtract)
                # in-place: var -> sd
                nc.scalar.activation(var[:, 0:w], var[:, 0:w], ACTF.Sqrt, bias=eps_t[0:1, :])
                # reciprocal emits bf16 directly: one fewer chain hop
                Ab_h = stp.tile([1, 512], BF16, tag="Abh", name=f"Abh_{b}_{h}_{m}_{toff}", bufs=3)
                Bb_h = stp.tile([1, 512], BF16, tag="Bbh", name=f"Bbh_{b}_{h}_{m}_{toff}", bufs=3)
                with nc.allow_low_precision(reason="LN scale in bf16; 0.4% on 2e-2 budget"):
                    nc.vector.reciprocal(Ab_h[:, 0:w], var[:, 0:w])
                    nc.vector.scalar_tensor_tensor(Bb_h[:, 0:w], mean[:, 0:w], -1.0, Ab_h[:, 0:w], ALU.mult, ALU.mult)
                return ys, Ab_h, Bb_h

            def passB(b, h, m, st, w=512, toff=0):
                t0 = h * TH + m * 512 + toff
                ys, Ab_h, Bb_h = st
                Ab = abp.tile([128, 512], BF16, tag="Ab")
                nc.gpsimd.partition_broadcast(Ab[:, 0:w], Ab_h[:, 0:w])
                yss = []
                for kt in range(4):
                    ysc = yp.tile([128, 512], BF16, tag="ysc", bufs=4)
                    nc.vector.tensor_tensor(ysc[:, 0:w], ys[kt][:, 0:w], Ab[:, 0:w], ALU.mult)
                    yss.append(ysc)
                for ob in range(4):
                    p4 = ps4.tile([128, 512], F32, tag="p4")
                    for kt in range(4):
                        nc.tensor.matmul(
                            p4[:, 0:w], m_t[:, kt * 512 + ob * 128:kt * 512 + ob * 128 + 128],
                            yss[kt][:, 0:w], start=(kt == 0), stop=False)
                    nc.tensor.matmul(p4[:, 0:w], rows_t[0:1, ob * 128:ob * 128 + 128],
                                     Bb_h[:, 0:w], start=False, stop=True)
                    of = fp.tile([128, 512], F32, tag="of")
                    # c7 (constant per channel) rides the evac bias port
                    nc.scalar.activation(of[:, 0:w], p4[:, 0:w], ACTF.Identity,
                                         bias=col(5, ob), scale=1.0)
                    nc.gpsimd.dma_start(out=out[b, ob * 128:(ob + 1) * 128, t0:t0 + w], in_=of[:, 0:w])

            def dsr(b, h, carries=None, fine=False):
                if fine:
                    derot_fine(b, h)
                else:
                    derot(b, h)
                scan(b, h, carries)

            def phase2_row(b, look=1):
                # interleave passA/passB with `look`-unit lookahead so PE
                # stays fed during each unit's LN scalar chain
                units = [(h, m) for h in range(NH) for m in range(TH // 512)]
                st = {}
                for i, (h, m) in enumerate(units):
                    st[i] = passA(b, h, m)
                    if i >= look:
                        ph, pm_ = units[i - look]
                        passB(b, ph, pm_, st.pop(i - look))
                for i in range(len(units) - look, len(units)):
                    passB(b, *units[i], st.pop(i))

            # Emission = per-engine execution order (in-order queues).  Each
            # half's rerot runs right after its scan (carry snapshot unlocks
            # the in-place overwrite), and phase2 units zip between row-1
            # scan/rerot stages so the DVE stream never drains.
            U = [(h, m) for h in range(NH) for m in range(TH // 512)]
            load_bb_half(1)
            load_misc()
            # PE pstate warm-up: dummy matmuls during DMA lead-ins / rerot
            # waits keep the tensor engine clock ramped. wide=True spans long
            # waits with 512-cycle dummies (rhs contents are irrelevant).
            def pe_warm(n, wide=False):
                if wide:
                    warm = pst.tile([1, 512], F32, tag="pstat", name="warmw")
                    for _ in range(n):
                        nc.tensor.matmul(warm[:], ones_t[:, 0:1], brg_t[:, 0:512],
                                         start=True, stop=True)
                else:
                    warm = pst.tile([1, 128], F32, tag="pstat", name="warm")
                    for _ in range(n):
                        nc.tensor.matmul(warm[:], ones_t[:, 0:1], ones_t[:, 0:128],
                                         start=True, stop=True)

            pe_warm(55)
            mm1(0, 0)
            for blk in range(4):
                load_tables(blk, 0)
            dsr(0, 0, fine=True)
            car0 = carry_save(0)
            rerot(0, 0)
            mm1(0, 1)
            for blk in range(4):
                load_tables(blk, 1)
            load_bulk_weights()
            dsr(0, 1, car0)
            rerot(0, 1)
            mm1(1, 0)
            mm1(1, 1)
            st0 = {}
            st1 = {}
            st0[0] = passA(0, *U[0])
            st0[1] = passA(0, *U[1])
            passB(0, *U[0], st0.pop(0))
            dsr(1, 0)
            car1 = carry_save(1)
            rerot(1, 0)
            pe_warm(8, wide=True)
            st0[2] = passA(0, *U[2])
            pe_warm(35, wide=True)
            passB(0, *U[1], st0.pop(1))
            dsr(1, 1, car1)
            pe_warm(30, wide=True)
            st1[0] = passA(1, *U[0])
            st1[1] = passA(1, *U[1])
            rerot(1, 1)
            st0[3] = passA(0, *U[3])
            passB(0, *U[2], st0.pop(2))
            passB(0, *U[3], st0.pop(3))
            passB(1, *U[0], st1.pop(0))
            pe_warm(15, wide=True)
            st1[2] = passA(1, *U[2])
            passB(1, *U[1], st1.pop(1))
            st1[3] = passA(1, *U[3])
            passB(1, *U[2], st1.pop(2))
            passB(1, *U[3], st1.pop(3))

    nc.compile()
    return nc


def _consts(inputs):
    f64 = np.float64
    theta = np.exp(inputs["theta_log"].astype(np.float32)).astype(f64)
    rho = np.exp(-np.exp(inputs["nu_log"].astype(f64)))
    gamma = np.exp(inputs["gamma_log"].astype(f64))
    tt = np.arange(T, dtype=f64)
    ang = theta[:, None] * tt[None, :]
    Ctab = np.cos(ang)
    Stab = np.sin(ang)
    W1 = inputs["W1"].astype(f64)
    W2 = inputs["W2"].astype(f64)
    W1s = W1 * inputs["ln_scale"].astype(f64)[:, None]
    M = W1s @ W2
    c6 = M.sum(0)
    c7 = (inputs["ln_bias"].astype(f64) @ W1 + inputs["b1"].astype(f64)) @ W2 \
        + inputs["b2"].astype(f64)
    cols = {
        0: rho,
        1: inputs["br"].astype(f64) * gamma,
        2: inputs["bi"].astype(f64) * gamma,
        3: (inputs["cr"] - inputs["ci"]).astype(f64),
        4: c6,
        5: c7,
    }
    cst = np.zeros((128, 4 * 6), np.float32)
    for c, v in cols.items():
        for blk in range(4):
            cst[:, c * 4 + blk] = v[blk * 128:(blk + 1) * 128].astype(np.float32)
    rows = np.stack([c6, c7]).astype(np.float32)  # [2, O]
    return cst, gamma, M, Ctab, Stab, rows


def _shared_inputs(inputs):
    cst, gamma, M, Ctab, Stab, rows = _consts(inputs)
    bf = ml_dtypes.bfloat16
    g32 = gamma.astype(np.float32)
    return dict(
        Brg=(inputs["Br"] * g32[None, :]).astype(bf),
        Big=(inputs["Bi"] * g32[None, :]).astype(bf),
        Crt=inputs["Cr"].astype(bf),
        Cin=(-inputs["Ci"]).astype(bf),
        Mw=M.astype(np.float32).astype(bf),
        rows=rows.astype(bf),
        Ctab=Ctab.astype(np.float32).astype(bf),
        Stab=Stab.astype(np.float32).astype(bf),
        cst=cst)


def _core_inputs(inputs, i):
    bf = ml_dtypes.bfloat16
    sh = _shared_inputs(inputs)
    xt = np.ascontiguousarray(
        inputs["x"][NB * i:NB * (i + 1)].transpose(0, 2, 1)).astype(bf)
    return dict(x_t=xt, **sh)


def kernel(x, nu_log, theta_log, gamma_log, Br, br, Bi, bi,
           Cr, cr, Ci, ci, ln_scale, ln_bias, W1, b1, W2, b2):
    if "nc" not in _CACHE:
        _CACHE["nc"] = _build()
    nc = _CACHE["nc"]
    inputs = dict(x=x, nu_log=nu_log, theta_log=theta_log, gamma_log=gamma_log,
                  Br=Br, br=br, Bi=Bi, bi=bi, Cr=Cr, cr=cr, Ci=Ci, ci=ci,
                  ln_scale=ln_scale, ln_bias=ln_bias, W1=W1, b1=b1, W2=W2, b2=b2)
    sh = _shared_inputs(inputs)
    bf = ml_dtypes.bfloat16
    xt = np.ascontiguousarray(x.transpose(0, 2, 1)).astype(bf)  # [B, H, T]
    in_maps = []
    for i in range(NCORES):
        in_maps.append(dict(x_t=xt[NB * i:NB * (i + 1)], **sh))
    res = run_bass_kernel_spmd(nc, in_maps, core_ids=list(range(NCORES)))
    out = np.empty((B, T, O), np.float32)
    for i in range(NCORES):
        o = res.results[i]["out_t"]  # [NB, O, T]
        out[NB * i:NB * (i + 1)] = o.transpose(0, 2, 1)
    return out


# revision 116
# speedup vs baseline: 1.0175x; 1.0048x over previous
import sys
sys.path.insert(0, '/opt/trn_rl_repo')
import numpy as np
import ml_dtypes
import concourse.bacc as bacc
import concourse.mybir as mybir
import concourse.tile as tile
from concourse.bass_utils import run_bass_kernel_spmd

F32 = mybir.dt.float32
BF16 = mybir.dt.bfloat16
ALU = mybir.AluOpType
ACTF = mybir.ActivationFunctionType

B, T, H, O = 16, 2048, 512, 512
NB = 2            # batch rows per core
NCORES = 8
TH = 1024         # scan half length
NH = T // TH      # halves per row
LN_EPS = 1e-6

_CACHE = {}


def _build():
    nc = bacc.Bacc(None, target_bir_lowering=False)
    xin = nc.declare_dram_parameter("x_t", [NB, H, T], BF16, False)
    Brg = nc.declare_dram_parameter("Brg", [H, H], BF16, False)
    Big = nc.declare_dram_parameter("Big", [H, H], BF16, False)
    Crt = nc.declare_dram_parameter("Crt", [H, H], BF16, False)
    Cin = nc.declare_dram_parameter("Cin", [H, H], BF16, False)
    Mw = nc.declare_dram_parameter("Mw", [H, O], BF16, False)
    Rws = nc.declare_dram_parameter("rows", [2, O], BF16, False)
    Ctp = nc.declare_dram_parameter("Ctab", [H, T], BF16, False)
    Stp = nc.declare_dram_parameter("Stab", [H, T], BF16, False)
    CST = nc.declare_dram_parameter("cst", [128, 4 * 6], F32, False)
    out = nc.declare_dram_parameter("out_t", [NB, O, T], F32, True)

    with tile.TileContext(nc) as tc:
        with tc.tile_pool(name="wpool", bufs=1) as wp, \
             tc.tile_pool(name="upool", bufs=1) as up, \
             tc.tile_pool(name="xp", bufs=2) as xp, \
             tc.tile_pool(name="scr", bufs=1) as scr, \
             tc.tile_pool(name="yp", bufs=4) as yp, \
             tc.tile_pool(name="stp", bufs=1) as stp, \
             tc.tile_pool(name="abp", bufs=2) as abp, \
             tc.tile_pool(name="fp", bufs=2) as fp, \
             tc.tile_pool(name="ps_mm1", bufs=2, space="PSUM") as ps1, \
             tc.tile_pool(name="ps_c", bufs=2, space="PSUM") as ps2, \
             tc.tile_pool(name="ps_st", bufs=2, space="PSUM") as pst, \
             tc.tile_pool(name="ps_mlp", bufs=2, space="PSUM") as ps4:

            # ---- resident weights / constants ----
            brg_t = wp.tile([128, 4 * 512], BF16, tag="brg")
            big_t = wp.tile([128, 4 * 512], BF16, tag="big")
            cr_t = wp.tile([128, 4 * 512], BF16, tag="cr")
            ci_t = wp.tile([128, 4 * 512], BF16, tag="ci")
            m_t = wp.tile([128, 4 * 512], BF16, tag="m")
            c_t = wp.tile([128, 4 * T], BF16, tag="ctab")
            s_t = wp.tile([128, 4 * T], BF16, tag="stab")
            cst_t = wp.tile([128, 4 * 6], F32, tag="cst")
            rows_t = wp.tile([1, 2 * O], BF16, tag="rows")
            ones_t = wp.tile([128, 128], BF16, tag="ones")

            eps_t = wp.tile([128, 1], F32, tag="eps")
            def load_bb_half(half):
                # issue via the Act DGE queue so these overlap the SP-queue
                # x loads on the physical DMA engines
                for (dst, src) in ((brg_t, Brg), (big_t, Big)):
                    nc.scalar.dma_start(
                        out=dst[:, half * 1024:(half + 1) * 1024].rearrange(
                            "p (k n) -> p k n", k=2),
                        in_=src[half * 256:(half + 1) * 256, :].rearrange(
                            "(k p) n -> p k n", p=128))

            load_bb_half(0)
            nc.vector.memset(ones_t[:], 1.0)

            nc.vector.memset(eps_t[:], LN_EPS)

            def load_misc():
                nc.scalar.dma_start(out=cst_t[:], in_=CST[:])

            def load_tables(blk, th):
                # (blk, t-half) pieces on the Pool DGE queue: not gated on the
                # full transfers nor on the SP/Act queue backlogs
                for (dst, src) in ((c_t, Ctp), (s_t, Stp)):
                    nc.gpsimd.dma_start(
                        out=dst[:, blk * T + th * TH:blk * T + (th + 1) * TH],
                        in_=src[blk * 128:(blk + 1) * 128, th * TH:(th + 1) * TH])

            def load_bulk_weights():
                nc.sync.dma_start(
                    out=rows_t[:].rearrange("p (a n) -> p a n", a=2),
                    in_=Rws[:].rearrange("(p a) n -> p a n", p=1))
                for (dst, src) in ((cr_t, Crt), (ci_t, Cin), (m_t, Mw)):
                    nc.sync.dma_start(
                        out=dst[:].rearrange("p (k n) -> p k n", k=4),
                        in_=src[:].rearrange("(k p) n -> p k n", p=128))

            def col(c, blk):
                return cst_t[:, c * 4 + blk:c * 4 + blk + 1]

            # A: ping-pong U/V buffers (half-row granularity, in-place derot)
            Ar = up.tile([128, 2 * 4 * TH], BF16, tag="ar")
            Ai = up.tile([128, 2 * 4 * TH], BF16, tag="ai")
            avr = Ar[:].rearrange("p (q k t) -> p q k t", q=2, k=4)
            avi = Ai[:].rearrange("p (q k t) -> p q k t", q=2, k=4)
            # B: G -> H (in-place rerot), full rows
            Gr = up.tile([128, 4 * NB * T], BF16, tag="gr")
            Gi = up.tile([128, 4 * NB * T], BF16, tag="gi")
            b4r = Gr[:].rearrange("p (k b t) -> p k b t", k=4, b=NB)
            b4i = Gi[:].rearrange("p (k b t) -> p k b t", k=4, b=NB)
            c4 = c_t[:].rearrange("p (k t) -> p k t", k=4)
            s4 = s_t[:].rearrange("p (k t) -> p k t", k=4)

            def asl(av, pp, blk, c0, c1):
                return av[:, pp:pp + 1, blk:blk + 1, c0:c1].squeeze()

            def bsl(b4, blk, b, c0, c1):
                return b4[:, blk:blk + 1, b:b + 1, c0:c1].squeeze()

            def csl(c4v, blk, c0, c1):
                return c4v[:, blk:blk + 1, c0:c1].squeeze()

            def mm1(b, h):
                # ob-major loop order: each h-block's U completes early so the
                # derot for blk 0 starts after ~1/4 of mm1, not all of it
                pp = (2 * b + h) % 2
                xts = []
                for m in range(TH // 512):
                    t0 = h * TH + m * 512
                    xt = xp.tile([128, 4 * 512], BF16, tag="xt")
                    for half in range(2):
                        nc.sync.dma_start(
                            out=xt[:, half * 1024:(half + 1) * 1024].rearrange(
                                "p (k t) -> p k t", k=2),
                            in_=xin[b, half * 256:(half + 1) * 256, t0:t0 + 512].rearrange(
                                "(k p) t -> p k t", p=128))
                    xts.append(xt)
                for ob in range(4):
                    for m in range(TH // 512):
                        for (wt, dst, bcol) in ((brg_t, avr, 1), (big_t, avi, 2)):
                            pm = ps1.tile([128, 512], F32, tag="pm1")
                            for kt in range(4):
                                nc.tensor.matmul(
                                    pm[:], wt[:, kt * 512 + ob * 128:kt * 512 + ob * 128 + 128],
                                    xts[m][:, kt * 512:(kt + 1) * 512],
                                    start=(kt == 0), stop=(kt == 3))
                            slab = asl(dst, pp, ob, m * 512, (m + 1) * 512)
                            nc.scalar.activation(
                                slab, pm[:], ACTF.Identity, bias=col(bcol, ob), scale=1.0)

            def derot_fine(b, h):
                # per-blk variant: blk k only needs ob-k's mm1 evacs, so the
                # first derot of the kernel starts ~3us earlier; blk 0 is
                # additionally split by m-tile (needs only ob0-m0 to begin)
                pp = (2 * b + h) % 2
                g0 = h * TH
                for blk in range(4):
                    segs = ((0, 512), (512, TH)) if blk == 0 else ((0, TH),)
                    for (c0, c1) in segs:
                        w = c1 - c0
                        ur = asl(avr, pp, blk, c0, c1)
                        ui = asl(avi, pp, blk, c0, c1)
                        cc = csl(c4, blk, g0 + c0, g0 + c1)
                        ss = csl(s4, blk, g0 + c0, g0 + c1)
                        t1 = scr.tile([128, 2 * TH], BF16, tag="sc1")
                        t2 = scr.tile([128, 2 * TH], BF16, tag="sc2")
                        nc.vector.tensor_tensor(t1[:, 0:w], ur, ss, ALU.mult)
                        nc.vector.tensor_tensor(t2[:, 0:w], ui, ss, ALU.mult)
                        nc.vector.tensor_tensor(ur, ur, cc, ALU.mult)
                        nc.vector.tensor_tensor(ui, ui, cc, ALU.mult)
                        nc.vector.tensor_tensor(ur, ur, t2[:, 0:w], ALU.add)
                        nc.vector.tensor_tensor(ui, ui, t1[:, 0:w], ALU.subtract)

            def derot(b, h):
                # in-place: U -> V = e^{-i theta t} U; blk-pair ops (3-D APs)
                pp = (2 * b + h) % 2
                g0 = h * TH
                for pr in range(2):
                    k0, k1 = 2 * pr, 2 * pr + 2
                    ur = avr[:, pp:pp + 1, k0:k1, :].squeeze()
                    ui = avi[:, pp:pp + 1, k0:k1, :].squeeze()
                    cc = c4[:, k0:k1, g0:g0 + TH]
                    ss = s4[:, k0:k1, g0:g0 + TH]
                    t1 = scr.tile([128, 2 * TH], BF16, tag="sc1")
                    t2 = scr.tile([128, 2 * TH], BF16, tag="sc2")
                    t1v = t1[:].rearrange("p (a t) -> p a t", a=2)
                    t2v = t2[:].rearrange("p (a t) -> p a t", a=2)
                    nc.vector.tensor_tensor(t1v, ur, ss, ALU.mult)
                    nc.vector.tensor_tensor(t2v, ui, ss, ALU.mult)
                    nc.vector.tensor_tensor(ur, ur, cc, ALU.mult)
                    nc.vector.tensor_tensor(ui, ui, cc, ALU.mult)
                    nc.vector.tensor_tensor(ur, ur, t2v, ALU.add)
                    nc.vector.tensor_tensor(ui, ui, t1v, ALU.subtract)

            def scan(b, h, carries):
                pp = (2 * b + h) % 2
                g0 = h * TH
                for blk in range(4):
                    vr = asl(avr, pp, blk, 0, TH)
                    vi = asl(avi, pp, blk, 0, TH)
                    gr = bsl(b4r, blk, b, g0, g0 + TH)
                    gi = bsl(b4i, blk, b, g0, g0 + TH)
                    rho = col(0, blk).broadcast_to([128, TH])
                    if h == 0:
                        init_r, init_i = 0.0, 0.0
                    else:
                        car_r, car_i = carries
                        init_r = car_r[:, blk:blk + 1]
                        init_i = car_i[:, blk:blk + 1]
                    nc.vector.tensor_tensor_scan(gr, rho, vr, init_r, ALU.mult, ALU.add)
                    nc.vector.tensor_tensor_scan(gi, rho, vi, init_i, ALU.mult, ALU.add)

            def carry_save(b):
                # snapshot G[:, TH-1] per blk before rerot(b,0) clobbers it
                car_r = stp.tile([128, 4], BF16, tag="carR", bufs=2, name=f"carR_{b}")
                car_i = stp.tile([128, 4], BF16, tag="carI", bufs=2, name=f"carI_{b}")
                src_r = b4r[:, :, b:b + 1, TH - 1:TH].squeeze()
                src_i = b4i[:, :, b:b + 1, TH - 1:TH].squeeze()
                nc.vector.tensor_copy(car_r[:], src_r)
                nc.vector.tensor_copy(car_i[:], src_i)
                return car_r, car_i

            def rerot(b, h):
                # in-place: G -> H = e^{+i theta t} G; blk-pair ops (3-D APs)
                g0 = h * TH
                for pr in range(2):
                    k0, k1 = 2 * pr, 2 * pr + 2
                    gr = b4r[:, k0:k1, b:b + 1, g0:g0 + TH].squeeze()
                    gi = b4i[:, k0:k1, b:b + 1, g0:g0 + TH].squeeze()
                    cc = c4[:, k0:k1, g0:g0 + TH]
                    ss = s4[:, k0:k1, g0:g0 + TH]
                    t1 = scr.tile([128, 2 * TH], BF16, tag="sc1")
                    t2 = scr.tile([128, 2 * TH], BF16, tag="sc2")
                    t1v = t1[:].rearrange("p (a t) -> p a t", a=2)
                    t2v = t2[:].rearrange("p (a t) -> p a t", a=2)
                    nc.vector.tensor_tensor(t1v, gr, ss, ALU.mult)
                    nc.vector.tensor_tensor(t2v, gi, ss, ALU.mult)
                    nc.vector.tensor_tensor(gr, gr, cc, ALU.mult)
                    nc.vector.tensor_tensor(gi, gi, cc, ALU.mult)
                    nc.vector.tensor_tensor(gr, gr, t2v, ALU.subtract)
                    nc.vector.tensor_tensor(gi, gi, t1v, ALU.add)

            def passA(b, h, m, w=512, toff=0):
                # C readout + LN stats chain; returns tiles passB needs
                t0 = h * TH + m * 512 + toff
                ys = []
                for ob in range(4):
                    p2 = ps2.tile([128, 512], F32, tag="p2")
                    for kt in range(4):
                        nc.tensor.matmul(
                            p2[:, 0:w], cr_t[:, kt * 512 + ob * 128:kt * 512 + ob * 128 + 128],
                            bsl(b4r, kt, b, t0, t0 + w), start=(kt == 0), stop=False)
                    for kt in range(4):
                        nc.tensor.matmul(
                            p2[:, 0:w], ci_t[:, kt * 512 + ob * 128:kt * 512 + ob * 128 + 128],
                            bsl(b4i, kt, b, t0, t0 + w), start=False, stop=(kt == 3))
                    y = yp.tile([128, 512], BF16, tag="y", bufs=16)
                    nc.scalar.activation(y[:, 0:w], p2[:, 0:w], ACTF.Identity, bias=col(3, ob), scale=1.0)
                    ys.append(y)
                y2s = []
                for ob in range(4):
                    y2 = yp.tile([128, 512], BF16, tag="y2", bufs=2)
                    nc.scalar.activation(y2[:, 0:w], ys[ob][:, 0:w], ACTF.Square)
                    y2s.append(y2)
                s1 = pst.tile([1, 512], F32, tag="pstat", name="s1")
                s2 = pst.tile([1, 512], F32, tag="pstat", name="s2")
                for ob in range(4):
                    nc.tensor.matmul(s1[:, 0:w], ones_t[:, 0:1], ys[ob][:, 0:w],
                                     start=(ob == 0), stop=(ob == 3))
                for ob in range(4):
                    nc.tensor.matmul(s2[:, 0:w], ones_t[:, 0:1], y2s[ob][:, 0:w],
                                     start=(ob == 0), stop=(ob == 3))
                mean = stp.tile([1, 512], F32, tag="mean")
                ms = stp.tile([1, 512], F32, tag="ms")
                var = stp.tile([1, 512], F32, tag="var")
                nc.scalar.activation(mean[:, 0:w], s1[:, 0:w], ACTF.Copy, scale=1.0 / H)
                nc.scalar.activation(ms[:, 0:w], s1[:, 0:w], ACTF.Square, scale=1.0 / H)
                nc.vector.scalar_tensor_tensor(var[:, 0:w], s2[:, 0:w], 1.0 / H, ms[:, 0:w], ALU.mult, ALU.subtract)
                # in-place: var -> sd
                nc.scalar.activation(var[:, 0:w], var[:, 0:w], ACTF.Sqrt, bias=eps_t[0:1, :])
                # reciprocal emits bf16 directly: one fewer chain hop
                Ab_h = stp.tile([1, 512], BF16, tag="Abh", name=f"Abh_{b}_{h}_{m}_{toff}", bufs=3)
                Bb_h = stp.tile([1, 512], BF16, tag="Bbh", name=f"Bbh_{b}_{h}_{m}_{toff}", bufs=3)
                with nc.allow_low_precision(reason="LN scale in bf16; 0.4% on 2e-2 budget"):
                    nc.vector.reciprocal(Ab_h[:, 0:w], var[:, 0:w])
                    nc.vector.scalar_tensor_tensor(Bb_h[:, 0:w], mean[:, 0:w], -1.0, Ab_h[:, 0:w], ALU.mult, ALU.mult)
                return ys, Ab_h, Bb_h

            def passB(b, h, m, st, w=512, toff=0):
                t0 = h * TH + m * 512 + toff
                ys, Ab_h, Bb_h = st
                Ab = abp.tile([128, 512], BF16, tag="Ab")
                nc.gpsimd.partition_broadcast(Ab[:, 0:w], Ab_h[:, 0:w])
                yss = []
                for kt in range(4):
                    ysc = yp.tile([128, 512], BF16, tag="ysc", bufs=4)
                    nc.vector.tensor_tensor(ysc[:, 0:w], ys[kt][:, 0:w], Ab[:, 0:w], ALU.mult)
                    yss.append(ysc)
                for ob in range(4):
                    p4 = ps4.tile([128, 512], F32, tag="p4")
                    for kt in range(4):
                        nc.tensor.matmul(
                            p4[:, 0:w], m_t[:, kt * 512 + ob * 128:kt * 512 + ob * 128 + 128],
                            yss[kt][:, 0:w], start=(kt == 0), stop=False)
                    nc.tensor.matmul(p4[:, 0:w], rows_t[0:1, ob * 128:ob * 128 + 128],
                                     Bb_h[:, 0:w], start=False, stop=True)
                    of = fp.tile([128, 512], F32, tag="of")
                    # c7 (constant per channel) rides the evac bias port
                    nc.scalar.activation(of[:, 0:w], p4[:, 0:w], ACTF.Identity,
                                         bias=col(5, ob), scale=1.0)
                    nc.gpsimd.dma_start(out=out[b, ob * 128:(ob + 1) * 128, t0:t0 + w], in_=of[:, 0:w])

            def dsr(b, h, carries=None, fine=False):
                if fine:
                    derot_fine(b, h)
                else:
                    derot(b, h)
                scan(b, h, carries)

            def phase2_row(b, look=1):
                # interleave passA/passB with `look`-unit lookahead so PE
                # stays fed during each unit's LN scalar chain
                units = [(h, m) for h in range(NH) for m in range(TH // 512)]
                st = {}
                for i, (h, m) in enumerate(units):
                    st[i] = passA(b, h, m)
                    if i >= look:
                        ph, pm_ = units[i - look]
                        passB(b, ph, pm_, st.pop(i - look))
                for i in range(len(units) - look, len(units)):
                    passB(b, *units[i], st.pop(i))

            # Emission = per-engine execution order (in-order queues).  Each
            # half's rerot runs right after its scan (carry snapshot unlocks
            # the in-place overwrite), and phase2 units zip between row-1
            # scan/rerot stages so the DVE stream never drains.
            U = [(h, m) for h in range(NH) for m in range(TH // 512)]
            load_bb_half(1)
            load_misc()
            # PE pstate warm-up: dummy matmuls during DMA lead-ins / rerot
            # waits keep the tensor engine clock ramped. wide=True spans long
            # waits with 512-cycle dummies (rhs contents are irrelevant).
            def pe_warm(n, wide=False):
                if wide:
                    warm = pst.tile([1, 512], F32, tag="pstat", name="warmw")
                    for _ in range(n):
                        nc.tensor.matmul(warm[:], ones_t[:, 0:1], brg_t[:, 0:512],
                                         start=True, stop=True)
                else:
                    warm = pst.tile([1, 128], F32, tag="pstat", name="warm")
                    for _ in range(n):
                        nc.tensor.matmul(warm[:], ones_t[:, 0:1], ones_t[:, 0:128],
                                         start=True, stop=True)

            pe_warm(55)
            mm1(0, 0)
            for blk in range(4):
                load_tables(blk, 0)
            dsr(0, 0, fine=True)
            car0 = carry_save(0)
            rerot(0, 0)
            mm1(0, 1)
            for blk in range(4):
                load_tables(blk, 1)
            load_bulk_weights()
            dsr(0, 1, car0)
            rerot(0, 1)
            mm1(1, 0)
            mm1(1, 1)
            st0 = {}
            st1 = {}
            st0[0] = passA(0, *U[0])
            st0[1] = passA(0, *U[1])
            passB(0, *U[0], st0.pop(0))
            dsr(1, 0)
            car1 = carry_save(1)
            rerot(1, 0)
            pe_warm(8, wide=True)
            st0[2] = passA(0, *U[2])
            pe_warm(35, wide=True)
            passB(0, *U[1], st0.pop(1))
            dsr(1, 1, car1)
            pe_warm(30, wide=True)
            st1[0] = passA(1, *U[0])
            st1[1] = passA(1, *U[1])
            rerot(1, 1)
            st0[3] = passA(0, *U[3])
            passB(0, *U[2], st0.pop(2))
            passB(0, *U[3], st0.pop(3))
            passB(1, *U[0], st1.pop(0))
            pe_warm(15, wide=True)
            st1[2] = passA(1, *U[2])
            passB(1, *U[1], st1.pop(1))
            st1[3] = passA(1, *U[3])
            passB(1, *U[2], st1.pop(2))
            passB(1, *U[3], st1.pop(3))

    nc.compile()
    return nc


def _consts(inputs):
    f64 = np.float64
    theta = np.exp(inputs["theta_log"].astype(np.float32)).astype(f64)
    rho = np.exp(-np.exp(inputs["nu_log"].astype(f64)))
    gamma = np.exp(inputs["gamma_log"].astype(f64))
    tt = np.arange(T, dtype=f64)
    ang = theta[:, None] * tt[None, :]
    Ctab = np.cos(ang)
    Stab = np.sin(ang)
    W1 = inputs["W1"].astype(f64)
    W2 = inputs["W2"].astype(f64)
    W1s = W1 * inputs["ln_scale"].astype(f64)[:, None]
    M = W1s @ W2
    c6 = M.sum(0)
    c7 = (inputs["ln_bias"].astype(f64) @ W1 + inputs["b1"].astype(f64)) @ W2 \
        + inputs["b2"].astype(f64)
    cols = {
        0: rho,
        1: inputs["br"].astype(f64) * gamma,
        2: inputs["bi"].astype(f64) * gamma,
        3: (inputs["cr"] - inputs["ci"]).astype(f64),
        4: c6,
        5: c7,
    }
    cst = np.zeros((128, 4 * 6), np.float32)
    for c, v in cols.items():
        for blk in range(4):
            cst[:, c * 4 + blk] = v[blk * 128:(blk + 1) * 128].astype(np.float32)
    rows = np.stack([c6, c7]).astype(np.float32)  # [2, O]
    return cst, gamma, M, Ctab, Stab, rows


def _shared_inputs(inputs):
    cst, gamma, M, Ctab, Stab, rows = _consts(inputs)
    bf = ml_dtypes.bfloat16
    g32 = gamma.astype(np.float32)
    return dict(
        Brg=(inputs["Br"] * g32[None, :]).astype(bf),
        Big=(inputs["Bi"] * g32[None, :]).astype(bf),
        Crt=inputs["Cr"].astype(bf),
        Cin=(-inputs["Ci"]).astype(bf),
        Mw=M.astype(np.float32).astype(bf),
        rows=rows.astype(bf),
        Ctab=Ctab.astype(np.float32).astype(bf),
        Stab=Stab.astype(np.float32).astype(bf),
        cst=cst)


def _core_inputs(inputs, i):
    bf = ml_dtypes.bfloat16
    sh = _shared_inputs(inputs)
    xt = np.ascontiguousarray(
        inputs["x"][NB * i:NB * (i + 1)].transpose(0, 2, 1)).astype(bf)
    return dict(x_t=xt, **sh)


def kernel(x, nu_log, theta_log, gamma_log, Br, br, Bi, bi,
           Cr, cr, Ci, ci, ln_scale, ln_bias, W1, b1, W2, b2):
    if "nc" not in _CACHE:
        _CACHE["nc"] = _build()
    nc = _CACHE["nc"]
    inputs = dict(x=x, nu_log=nu_log, theta_log=theta_log, gamma_log=gamma_log,
                  Br=Br, br=br, Bi=Bi, bi=bi, Cr=Cr, cr=cr, Ci=Ci, ci=ci,
                  ln_scale=ln_scale, ln_bias=ln_bias, W1=W1, b1=b1, W2=W2, b2=b2)
    sh = _shared_inputs(inputs)
    bf = ml_dtypes.bfloat16
    xt = np.ascontiguousarray(x.transpose(0, 2, 1)).astype(bf)  # [B, H, T]
    in_maps = []
    for i in range(NCORES):
        in_maps.append(dict(x_t=xt[NB * i:NB * (i + 1)], **sh))
    res = run_bass_kernel_spmd(nc, in_maps, core_ids=list(range(NCORES)))
    out = np.empty((B, T, O), np.float32)
    for i in range(NCORES):
        o = res.results[i]["out_t"]  # [NB, O, T]
        out[NB * i:NB * (i + 1)] = o.transpose(0, 2, 1)
    return out
